# revision 36
# baseline (speedup 1.0000x reference)
"""Trainium2 Bass kernel for nn_CandidateFinder (LSH hash-equality KNN).

Reference semantics: q/k binarized (x>0), projected by W [64,8], sign bits
packed into an 8-bit bucket code; for each query, return the first 64 key
indices (ascending) whose code equals the query's code, padded with -1.

Key insight: codes live in [0,256). Build, per batch, a [256, 64] table of
the first 64 key indices per bucket, then gather per query. Both steps map
onto matmuls + a free-dim prefix scan + GPSIMD local_scatters.

Sharding: 8 cores = 4 batches x 2 bucket-halves (c in [0,128) / [128,256)).
Each core computes a partial gather (zero where the query's code is in the
other half); host sums the pair and subtracts 1 (table stores j+1, empty=0).

Pipeline per core (engines balanced; two 512-key chunks share each hash
matmul by stacking them on partitions 0:64/64:128 against block-diagonal
weights, so one [128,512] matmul pair hashes 1024 keys):
  k: bin (DVE, stacked) -> paired hash mms (PE; first pair split into
     column-halves) -> bits: pair 1 via ACT Sign halves (+-1, Relu bias
     -7), pair 2 via DVE is_gt (0/1, Relu bias 1-popcount) -> agree mms
     (PE) -> Relu (ACT) -> prefix scan + mask, chunk 0 in eighths then
     quarters (DVE) -> local_scatter per quarter (Pool) -> merges (Pool)
  q: bin (Pool, stacked) -> paired hash mms (PE) -> Sign +-1 (ACT)
     -> agree mms (PE) -> one-hot: 3 chunks Relu (ACT) + 1 chunk is_gt
     (DVE, bf16 psum view, strided gather lhsT)
  out: 16 block matmuls vs the table (PE) -> psum copies (DVE+ACT)
     -> 2 DMAs (SP+ACT queues); host un-permutes the block layout
add_dep hints pin the PE order (early k agrees before the q hash) and the
DVE tail (scan chain before the q one-hot), which the list scheduler
otherwise inverts.

Measured critical path (CoreSim, 14026 ns/core): kT arrives ~2.7us (fixed
DMA latency); the scan chain starts at 4.82us, gated simultaneously by
the DVE prefix (bins + pair-2 is_gt; PSUM operands get no DVE speedup
modes) at 4819 and the hash->sign->agree->relu path at 4815; the
scan/mask/scatter chain is dependency-dense to 9.9us; gather, copies and
the two output DMAs finish issuing by 11.6us; the rest is fixed DMA
completion (~1.7us) plus the final barrier (~0.7us). Probed and rejected:
GPSIMD scan offload (Pool becomes the bottleneck), fp8 inputs/weights
(sign flips vs the f32 reference), finer DMA/gather splits (PSUM bank
budget and queue serialization), and all queue permutations for the
three input DMAs (each trades into a larger stall on a racing path).

Precision: the hash sign test needs ~f32-accurate projections. W is split
as fp16(W) + fp16(W - fp16(W)) and the two fp16 matmuls accumulate in f32
PSUM; representation error ~1e-6 vs hash sign margins ~1e-4 on this data.

k-side agree trick on 0/1 bits: #agreeing bits = pm^T bits + (8 - pop(c))
with pm = +-1 bit pattern of bucket c, so onehot = Relu(pm^T bits + bias_c),
bias_c = 1 - pop(c). q-side agree on +-1 signs: onehot = Relu(pm^T s - 7).

Scan mask: m1 = onehot*rank is the 1-based rank at matches (0 elsewhere);
idx = m1 - 1 is the 0-based slot at matches and -1 (ignored) elsewhere.
Tables are fp16 (iota data j+1; integers <= 2048 are fp16-exact) because
the Pool engine cannot add int16.
"""

import numpy as np
import ml_dtypes

B, L, D, NH = 4, 2048, 64, 8
KMAX = 64
TABLE_ELEMS = 256   # > max bucket count (90 on this data); idx beyond -> never
MPAD = 40           # hash matmul lhsT free size: 8 real + 32 zero rows
HALF = L // 2
QTR = L // 4

_cache = {}


def _build_program():
    import concourse.bass as bass
    import concourse.mybir as mybir
    from concourse import bacc, tile
    from contextlib import ExitStack

    dt = mybir.dt
    Alu = mybir.AluOpType
    Act = mybir.ActivationFunctionType

    nc = bacc.Bacc("TRN2", target_bir_lowering=False, debug=False)

    # DRAM I/O (per-core shapes)
    qT_d = nc.declare_dram_parameter("qT", [D, L], dt.bfloat16, isOutput=False)
    kT0_d = nc.declare_dram_parameter("kT0", [D, HALF], dt.bfloat16, isOutput=False)
    kT1_d = nc.declare_dram_parameter("kT1", [D, HALF], dt.bfloat16, isOutput=False)
    # block-diagonal hash weights: two chunks share one matmul by stacking
    # them on partitions 0:64 / 64:128. Columns 0:128 = [Whi-pad | Whi-pad]
    # block-diagonal (A window rows 0:40 at cols 0:40, B window rows 64:104
    # at cols 64:104, zero elsewhere); columns 128:256 = same for Wlo.
    wpk2_d = nc.declare_dram_parameter("wpk2", [128, 256], dt.float16, isOutput=False)
    # pm (+-1 bit patterns) at partition rows 0..8 and 64..72
    sgnc_d = nc.declare_dram_parameter("sgnc", [128, 128], dt.float16, isOutput=False)
    biask_d = nc.declare_dram_parameter("biask", [128, 2], dt.float32, isOutput=False)
    out_d = nc.declare_dram_parameter("out", [L, KMAX], dt.float16, isOutput=True)

    with ExitStack() as ctx:
        tc = ctx.enter_context(tile.TileContext(nc))
        sb = ctx.enter_context(tc.tile_pool(name="sb", bufs=1))
        hp = ctx.enter_context(tc.tile_pool(name="hp", bufs=3, space="PSUM"))
        ap = ctx.enter_context(tc.tile_pool(name="ap", bufs=3, space="PSUM"))
        gp = ctx.enter_context(tc.tile_pool(name="gp", bufs=1, space="PSUM"))

        # ---- loads: kT halves first (two queues), qT next, consts on ACT --
        kT0_sb = sb.tile([D, HALF], dt.bfloat16, tag="kT0")
        nc.sync.dma_start(kT0_sb[:], kT0_d[:])
        kT1_sb = sb.tile([D, HALF], dt.bfloat16, tag="kT1")
        nc.gpsimd.dma_start(kT1_sb[:], kT1_d[:])
        wpk2_sb = sb.tile([128, 256], dt.float16, tag="wpk2")
        nc.sync.dma_start(wpk2_sb[:], wpk2_d[:])
        qT_sb = sb.tile([D, L], dt.bfloat16, tag="qT")
        nc.sync.dma_start(qT_sb[:, 0:HALF], qT_d[:, 0:HALF])
        nc.gpsimd.dma_start(qT_sb[:, HALF:L], qT_d[:, HALF:L])
        sgnc_sb = sb.tile([128, 128], dt.float16, tag="sgnc")
        nc.scalar.dma_start(sgnc_sb[:], sgnc_d[:])
        biask_sb = sb.tile([128, 2], dt.float32, tag="biask")
        nc.scalar.dma_start(biask_sb[:], biask_d[:])

        # hash psum tiles: one [128, 512] tile per pair of chunks; chunk
        # 2g sits in the partition-block 0:64 and chunk 2g+1 in 64:128 of
        # the stacked input tile, and the block-diagonal wpk2 computes both
        # hashes in a single matmul pair (hi + lo): chunk 2g bits land at
        # psum rows 0:8, chunk 2g+1 bits at rows 64:72.
        hpk1a = hp.tile([128, 256], dt.float32, tag="hp", name="hpk1a")
        hpk1b = hp.tile([128, 256], dt.float32, tag="hp", name="hpk1b")
        hpk2 = hp.tile([128, 512], dt.float32, tag="hp", name="hpk2")

        # PE warm-up: anchor the p-state clock (a >~3us idle resets the PE
        # ramp). Garbage results land in hpk[0] rows that the real hash
        # matmuls overwrite with start=True.
        warm_sb = sb.tile([D, 64], dt.float16, tag="warm")
        nc.vector.memset(warm_sb[:], 0.0)
        for _ in range(2):
            nc.tensor.matmul(
                hpk1a[0:32, 0:64], lhsT=warm_sb[:, 0:32], rhs=warm_sb[:],
                start=True, stop=True,
            )

        def hash_pair(hpt, x2_ap):
            # x2 [128, n]: rows 0:64 = even chunk, 64:128 = odd chunk
            mm_hi = nc.tensor.matmul(
                hpt[:], lhsT=wpk2_sb[:, 0:128], rhs=x2_ap,
                start=True, stop=False,
            )
            nc.tensor.matmul(
                hpt[:], lhsT=wpk2_sb[:, 128:256], rhs=x2_ap,
                start=False, stop=True,
            )
            return mm_hi

        # ---- k side: bin (DVE) -> hash -> isgt 0/1 (DVE) -> agree ----
        xk2 = [
            sb.tile([128, 512], dt.float16, tag=f"xk2{g}", name=f"xk2{g}")
            for g in range(2)
        ]
        nc.vector.tensor_single_scalar(xk2[0][0:64, :], kT0_sb[:, 0:QTR], 0.0, Alu.is_gt)
        nc.vector.tensor_single_scalar(xk2[0][64:128, :], kT0_sb[:, QTR:HALF], 0.0, Alu.is_gt)
        nc.vector.tensor_single_scalar(xk2[1][0:64, :], kT1_sb[:, 0:QTR], 0.0, Alu.is_gt)
        nc.vector.tensor_single_scalar(xk2[1][64:128, :], kT1_sb[:, QTR:HALF], 0.0, Alu.is_gt)
        hash_pair(hpk1a, xk2[0][:, 0:256])
        hash_pair(hpk1b, xk2[0][:, 256:512])
        hash_pair(hpk2, xk2[1])

        # pair-1 bits as +-1 via ACT Sign in column-halves (ACT is free this
        # early; lets agree/relu/scan start sooner); pair-2 bits 0/1 via DVE
        s01k = sb.tile([128, 1024], dt.float16, tag="s01k")
        nc.scalar.activation(s01k[:, 0:256], hpk1a[:], Act.Sign)
        nc.scalar.activation(s01k[:, 256:512], hpk1b[:], Act.Sign)
        nc.vector.tensor_single_scalar(s01k[:, 512:1024], hpk2[:], 0.0, Alu.is_gt)

        # ---- q side: bin (Pool) -> hash -> sign +-1 (ACT halves) -> agree --
        xq2 = [
            sb.tile([128, 512], dt.float16, tag=f"xq2{g}", name=f"xq2{g}")
            for g in range(2)
        ]
        nc.gpsimd.tensor_single_scalar(xq2[0][0:64, :], qT_sb[:, 0:QTR], 0.0, Alu.is_gt)
        nc.gpsimd.tensor_single_scalar(xq2[0][64:128, :], qT_sb[:, QTR:HALF], 0.0, Alu.is_gt)
        nc.gpsimd.tensor_single_scalar(xq2[1][0:64, :], qT_sb[:, HALF : HALF + QTR], 0.0, Alu.is_gt)
        nc.gpsimd.tensor_single_scalar(xq2[1][64:128, :], qT_sb[:, HALF + QTR : L], 0.0, Alu.is_gt)

        # scatter data: each partition holds 0..L-1 (int16), off the
        # critical path (scat c0 needs it only after the first scan+mask)
        iota_sb = sb.tile([128, L], dt.float16, tag="iota")
        nc.gpsimd.iota(
            iota_sb[:], pattern=[[1, L]], base=1, channel_multiplier=0,
            allow_small_or_imprecise_dtypes=True,
        )

        onehot = sb.tile([128, L], dt.float16, tag="onehot")
        sq = sb.tile([128, 1024], dt.float16, tag="sq")
        q1h = sb.tile([128, 1536], dt.float16, tag="q1h")
        hpq = [hp.tile([128, 512], dt.float32, tag="hp", name=f"hpq{g}") for g in range(2)]

        def agree(rhs_ap, n, name):
            t = ap.tile([128, n], dt.float32, tag="apt", name=name)
            mm = nc.tensor.matmul(
                t[:],
                lhsT=sgnc_sb[rhs_ap.base_partition() : rhs_ap.base_partition() + 8, :],
                rhs=rhs_ap, start=True, stop=True,
            )
            return t, mm

        # PE emission order interleaves the q hash behind the k agrees so
        # the q chain keeps flowing while ACT works on the k relus.
        # k chunk 0 runs in column-halves so the scan chain starts early.
        from concourse.tile_rust import add_dep_helper

        apt_c0a, mm_c0a = agree(s01k[0:8, 0:256], 256, "apt_c0a")
        apt_c0b, mm_c0b = agree(s01k[0:8, 256:512], 256, "apt_c0b")
        apt_c1, mm_c1 = agree(s01k[64:72, 0:512], 512, "apt_c1")
        mm_hq0 = hash_pair(hpq[0], xq2[0])
        apt_c2, mm_c2 = agree(s01k[0:8, 512:1024], 512, "apt_c2")
        apt_c3, mm_c3 = agree(s01k[64:72, 512:1024], 512, "apt_c3")
        hash_pair(hpq[1], xq2[1])
        # keep the early k agrees (and the scan chain they feed) ahead of
        # everything later on PE
        add_dep_helper(mm_hq0.ins, mm_c1.ins, sync=False,
                       reason="chunk-0/1 agrees before q hash on PE")
        add_dep_helper(mm_c2.ins, mm_c1.ins, sync=False,
                       reason="chunk-0/1 agrees first on PE")

        # k one-hot: chunks 0,1 from +-1 signs (bias -7); 2,3 from 0/1 bits
        # (per-bucket bias 1-popcount)
        nc.scalar.activation(onehot[:, 0:256], apt_c0a[:], Act.Relu, bias=biask_sb[:, 1:2])
        nc.scalar.activation(onehot[:, 256:512], apt_c0b[:], Act.Relu, bias=biask_sb[:, 1:2])
        nc.scalar.activation(onehot[:, 512:1024], apt_c1[:], Act.Relu, bias=biask_sb[:, 1:2])
        nc.scalar.activation(onehot[:, 1024:1536], apt_c2[:], Act.Relu, bias=biask_sb[:, 0:1])
        nc.scalar.activation(onehot[:, 1536:2048], apt_c3[:], Act.Relu, bias=biask_sb[:, 0:1])
        nc.scalar.activation(sq[:, 0:512], hpq[0][:], Act.Sign)
        nc.scalar.activation(sq[:, 512:1024], hpq[1][:], Act.Sign)

        aptq = {}
        for u in range(4):
            r = 64 * (u % 2)
            g = u // 2
            aptq[u], _ = agree(sq[r : r + 8, 512 * g : 512 * (g + 1)], 512, f"aptq{u}")
        # q one-hot: chunks 0-2 on ACT, chunk 3 on DVE (after the scan chain)
        for u in range(3):
            nc.scalar.activation(
                q1h[:, 512 * u : 512 * (u + 1)], aptq[u][:],
                Act.Relu, bias=biask_sb[:, 1:2],
            )

        # ---- rank keys within bucket: quarter-granularity scan + mask.
        # m1 = onehot*rank is the 1-based rank at matches (0 elsewhere);
        # idx = m1 - 1 is the 0-based slot at matches, -1 (ignored) else.
        rank = sb.tile([128, L], dt.float16, tag="rank")
        m1 = sb.tile([128, L], dt.float16, tag="m1")
        idx16 = sb.tile([128, L], dt.int16, tag="idx16")
        # chunk 0 scans in eighths (earliest possible start); its mask runs
        # merged at [*, 512] after both eighth-scans (less total DVE work)
        scan_pieces = [(0, 256), (256, 512), (512, 1024), (1024, 1536), (1536, 2048)]
        mask_pieces = [(0, 512), (512, 1024), (1024, 1536), (1536, 2048)]
        scans = {}
        for lo, hi in scan_pieces:
            init = 0.0 if lo == 0 else rank[:, lo - 1 : lo]
            scans[lo] = nc.vector.tensor_tensor_scan(
                rank[:, lo:hi], onehot[:, lo:hi], onehot[:, lo:hi],
                init, Alu.add, Alu.bypass,
            )
        sub_inst = {}
        for i, (lo, hi) in enumerate(mask_pieces):
            nc.vector.tensor_mul(m1[:, lo:hi], onehot[:, lo:hi], rank[:, lo:hi])
            sub_inst[i] = nc.vector.tensor_single_scalar(
                idx16[:, lo:hi], m1[:, lo:hi], 1.0, Alu.subtract
            )
        tabs = []
        for c in range(4):
            lo, hi = QTR * c, QTR * (c + 1)
            tab = sb.tile([128, TABLE_ELEMS], dt.float16, tag=f"table{c}")
            tabs.append(tab)
            nc.gpsimd.local_scatter(
                tab[:], iota_sb[:, lo:hi], idx16[:, lo:hi],
                channels=128, num_elems=TABLE_ELEMS, num_idxs=QTR,
            )

        # q one-hot chunk 3 on DVE, held behind the scan chain so it does
        # not preempt the table build
        from concourse.tile_rust import add_dep_helper

        q1hx = sb.tile([128, 1024], dt.float16, tag="q1hx")
        aptq3_bf = aptq[3][:].bitcast(dt.bfloat16)
        q3_inst = nc.vector.tensor_single_scalar(q1hx[:], aptq3_bf, 7.0, Alu.is_gt)
        add_dep_helper(
            q3_inst.ins, sub_inst[3].ins, sync=False,
            reason="finish scan chain before q one-hot tail",
        )

        # merge quarter tables on Pool (disjoint nonzero slots); columns
        # 0..63 hold the first 64 matches (j+1) per bucket
        m01 = sb.tile([128, KMAX], dt.float16, tag="m01")
        nc.gpsimd.tensor_add(m01[:], tabs[0][:, 0:KMAX], tabs[1][:, 0:KMAX])
        m23 = sb.tile([128, KMAX], dt.float16, tag="m23")
        nc.gpsimd.tensor_add(m23[:], tabs[2][:, 0:KMAX], tabs[3][:, 0:KMAX])
        tab16 = sb.tile([128, KMAX], dt.float16, tag="tab16")
        nc.gpsimd.tensor_add(tab16[:], m01[:], m23[:])

        # ---- gather per query: out[i, s] = sum_c q1h[c, i] * tab16[c, s] ----
        # Chunk t takes queries {16p + t}, so psum partition p holds queries
        # 16p..16p+16 across chunks -> contiguous per-partition DRAM rows.
        q1hx_v = q1hx[:].rearrange("c (i two) -> c i two", two=2)[:, :, 1]
        HO = 8 * KMAX
        opA = gp.tile([128, HO], dt.float32, tag="gather", name="opA")
        opB = gp.tile([128, HO], dt.float32, tag="gatherB", name="opB")
        for t in range(16):
            dst = opA if t < 8 else opB
            if t < 12:
                lhsT = q1h[:, 128 * t : 128 * (t + 1)]
            else:
                lhsT = q1hx_v[:, 128 * (t - 12) : 128 * (t - 11)]
            nc.tensor.matmul(
                dst[:, KMAX * (t % 8) : KMAX * (t % 8 + 1)],
                lhsT=lhsT, rhs=tab16[:],
                start=True, stop=True,
            )
        out_v = out_d[:].rearrange("(p t) s -> p (t s)", p=128)  # [128, 1024] row-major view
        out0_sb = sb.tile([128, HO], dt.float16, tag="out0_sb")
        nc.vector.tensor_copy(out0_sb[:], opA[:])
        nc.sync.dma_start(out_v[:, 0:HO], out0_sb[:])
        out1_sb = sb.tile([128, HO], dt.float16, tag="out1_sb")
        nc.scalar.activation(out1_sb[:], opB[:], Act.Copy)
        nc.scalar.dma_start(out_v[:, HO : 2 * HO], out1_sb[:])

    nc.compile()
    return nc


def _get_nc():
    if "nc" not in _cache:
        _cache["nc"] = _build_program()
    return _cache["nc"]


def _make_in_maps(query, key, W):
    query = np.asarray(query, dtype=np.float32)
    key = np.asarray(key, dtype=np.float32)
    W = np.asarray(W, dtype=np.float32)
    qT = [
        np.ascontiguousarray(query[b].T).astype(ml_dtypes.bfloat16) for b in range(B)
    ]
    kT = [np.ascontiguousarray(key[b].T).astype(ml_dtypes.bfloat16) for b in range(B)]

    whi = W.astype(np.float16)
    wlo = (W - whi.astype(np.float32)).astype(np.float16)
    # block-diagonal pair weights: A window rows 0:64 -> cols 0:8 (+pad),
    # B window rows 64:128 -> cols 64:72; hi at cols 0:128, lo at 128:256
    wpk2 = np.zeros((128, 256), np.float16)
    wpk2[0:D, 0:NH] = whi
    wpk2[D : 2 * D, D : D + NH] = whi
    wpk2[0:D, 128 : 128 + NH] = wlo
    wpk2[D : 2 * D, 128 + D : 128 + D + NH] = wlo
    kT0 = [np.ascontiguousarray(kT[b][:, :HALF]) for b in range(B)]
    kT1 = [np.ascontiguousarray(kT[b][:, HALF:]) for b in range(B)]

    sgnc = []
    biask = []
    for h in range(2):
        cg = 128 * h + np.arange(128)  # global bucket ids of this half
        bits = ((cg[None, :] >> np.arange(NH)[:, None]) & 1).astype(np.float32)
        pm = (2.0 * bits - 1.0).astype(np.float16)  # [8, 128]
        arr = np.zeros((128, 128), np.float16)
        arr[0:NH] = pm
        arr[D : D + NH] = pm
        sgnc.append(arr)
        bk = np.empty((128, 2), np.float32)
        bk[:, 0] = 1.0 - bits.sum(axis=0)
        bk[:, 1] = -7.0
        biask.append(bk)
    return [
        {
            "qT": qT[c // 2],
            "kT0": kT0[c // 2],
            "kT1": kT1[c // 2],
            "wpk2": wpk2,
            "sgnc": sgnc[c % 2],
            "biask": biask[c % 2],
        }
        for c in range(2 * B)
    ]


def _combine(results):
    # device layout: [128, 16*64], partition p col t*64+s <-> query 128t+p
    out = np.empty((B, L, KMAX), dtype=np.int64)
    for b in range(B):
        g = results[2 * b]["out"].astype(np.int64) + results[2 * b + 1]["out"].astype(
            np.int64
        )
        g = g.reshape(128, 16, KMAX).transpose(1, 0, 2).reshape(L, KMAX)
        out[b] = g - 1
    return out


def _run_spmd(in_maps, **kwargs):
    from concourse.bass_utils import run_bass_kernel_spmd

    return run_bass_kernel_spmd(_get_nc(), in_maps, list(range(2 * B)), **kwargs)


def kernel(query, key, W, head_idx=0, **_unused):
    in_maps = _make_in_maps(query, key, W)
    res = _run_spmd(in_maps)
    return _combine(res.results)


# revision 37
# speedup vs baseline: 1.0147x; 1.0147x over previous
"""Trainium2 Bass kernel for nn_CandidateFinder (LSH hash-equality KNN).

Reference semantics: q/k binarized (x>0), projected by W [64,8], sign bits
packed into an 8-bit bucket code; for each query, return the first 64 key
indices (ascending) whose code equals the query's code, padded with -1.

Key insight: codes live in [0,256). Build, per batch, a [256, 64] table of
the first 64 key indices per bucket, then gather per query. Both steps map
onto matmuls + a free-dim prefix scan + GPSIMD local_scatters.

Sharding: 8 cores = 4 batches x 2 bucket-halves (c in [0,128) / [128,256)).
Each core computes a partial gather (zero where the query's code is in the
other half); host sums the pair and subtracts 1 (table stores j+1, empty=0).

Pipeline per core (engines balanced; two 512-key chunks share each hash
matmul by stacking them on partitions 0:64/64:128 against block-diagonal
weights, so one [128,512] matmul pair hashes 1024 keys):
  k: bin (DVE, stacked) -> paired hash mms (PE; first pair split into
     column-halves) -> bits: pair 1 via ACT Sign halves (+-1, Relu bias
     -7), pair 2 via DVE is_gt (0/1, Relu bias 1-popcount) -> agree mms
     (PE) -> Relu (ACT) -> prefix scan + mask, chunk 0 in eighths then
     quarters (DVE) -> local_scatter per quarter (Pool) -> merges (Pool)
  q: bin (Pool, stacked) -> paired hash mms (PE) -> Sign +-1 (ACT)
     -> agree mms (PE) -> one-hot: 3 chunks Relu (ACT) + 1 chunk is_gt
     (DVE, bf16 psum view, strided gather lhsT)
  out: 16 block matmuls vs the table (PE) -> psum copies (DVE+ACT)
     -> 2 DMAs (SP+ACT queues); host un-permutes the block layout
add_dep hints pin the PE order (early k agrees before the q hash) and the
DVE tail (scan chain before the q one-hot), which the list scheduler
otherwise inverts.

Measured critical path (CoreSim, 14026 ns/core): kT arrives ~2.7us (fixed
DMA latency); the scan chain starts at 4.82us, gated simultaneously by
the DVE prefix (bins + pair-2 is_gt; PSUM operands get no DVE speedup
modes) at 4819 and the hash->sign->agree->relu path at 4815; the
scan/mask/scatter chain is dependency-dense to 9.9us; gather, copies and
the two output DMAs finish issuing by 11.6us; the rest is fixed DMA
completion (~1.7us) plus the final barrier (~0.7us). Probed and rejected:
GPSIMD scan offload (Pool becomes the bottleneck), fp8 inputs/weights
(sign flips vs the f32 reference), finer DMA/gather splits (PSUM bank
budget and queue serialization), and all queue permutations for the
three input DMAs (each trades into a larger stall on a racing path).

Precision: the hash sign test needs ~f32-accurate projections. W is split
as fp16(W) + fp16(W - fp16(W)) and the two fp16 matmuls accumulate in f32
PSUM; representation error ~1e-6 vs hash sign margins ~1e-4 on this data.

k-side agree trick on 0/1 bits: #agreeing bits = pm^T bits + (8 - pop(c))
with pm = +-1 bit pattern of bucket c, so onehot = Relu(pm^T bits + bias_c),
bias_c = 1 - pop(c). q-side agree on +-1 signs: onehot = Relu(pm^T s - 7).

Scan mask: m1 = onehot*rank is the 1-based rank at matches (0 elsewhere);
idx = m1 - 1 is the 0-based slot at matches and -1 (ignored) elsewhere.
Tables are fp16 (iota data j+1; integers <= 2048 are fp16-exact) because
the Pool engine cannot add int16.
"""

import numpy as np
import ml_dtypes

B, L, D, NH = 4, 2048, 64, 8
KMAX = 64
TABLE_ELEMS = 256   # > max bucket count (90 on this data); idx beyond -> never
MPAD = 40           # hash matmul lhsT free size: 8 real + 32 zero rows
HALF = L // 2
QTR = L // 4

_cache = {}


def _build_program():
    import concourse.bass as bass
    import concourse.mybir as mybir
    from concourse import bacc, tile
    from contextlib import ExitStack

    dt = mybir.dt
    Alu = mybir.AluOpType
    Act = mybir.ActivationFunctionType

    nc = bacc.Bacc("TRN2", target_bir_lowering=False, debug=False)

    # DRAM I/O (per-core shapes)
    qT_d = nc.declare_dram_parameter("qT", [D, L], dt.bfloat16, isOutput=False)
    kT0_d = nc.declare_dram_parameter("kT0", [D, HALF], dt.bfloat16, isOutput=False)
    kT1_d = nc.declare_dram_parameter("kT1", [D, HALF], dt.bfloat16, isOutput=False)
    # block-diagonal hash weights: two chunks share one matmul by stacking
    # them on partitions 0:64 / 64:128. Columns 0:128 = [Whi-pad | Whi-pad]
    # block-diagonal (A window rows 0:40 at cols 0:40, B window rows 64:104
    # at cols 64:104, zero elsewhere); columns 128:256 = same for Wlo.
    wpk2_d = nc.declare_dram_parameter("wpk2", [128, 256], dt.float16, isOutput=False)
    # pm (+-1 bit patterns) at partition rows 0..8 and 64..72
    sgnc_d = nc.declare_dram_parameter("sgnc", [128, 128], dt.float16, isOutput=False)
    biask_d = nc.declare_dram_parameter("biask", [128, 2], dt.float32, isOutput=False)
    out_d = nc.declare_dram_parameter("out", [L, KMAX], dt.float16, isOutput=True)

    with ExitStack() as ctx:
        tc = ctx.enter_context(tile.TileContext(nc))
        sb = ctx.enter_context(tc.tile_pool(name="sb", bufs=1))
        hp = ctx.enter_context(tc.tile_pool(name="hp", bufs=3, space="PSUM"))
        ap = ctx.enter_context(tc.tile_pool(name="ap", bufs=3, space="PSUM"))
        gp = ctx.enter_context(tc.tile_pool(name="gp", bufs=1, space="PSUM"))

        # ---- loads: kT halves first (two queues), qT next, consts on ACT --
        kT0_sb = sb.tile([D, HALF], dt.bfloat16, tag="kT0")
        nc.sync.dma_start(kT0_sb[:], kT0_d[:])
        kT1_sb = sb.tile([D, HALF], dt.bfloat16, tag="kT1")
        nc.gpsimd.dma_start(kT1_sb[:], kT1_d[:])
        wpk2_sb = sb.tile([128, 256], dt.float16, tag="wpk2")
        nc.sync.dma_start(wpk2_sb[:], wpk2_d[:])
        qT_sb = sb.tile([D, L], dt.bfloat16, tag="qT")
        nc.sync.dma_start(qT_sb[:, 0:HALF], qT_d[:, 0:HALF])
        nc.gpsimd.dma_start(qT_sb[:, HALF:L], qT_d[:, HALF:L])
        sgnc_sb = sb.tile([128, 128], dt.float16, tag="sgnc")
        nc.scalar.dma_start(sgnc_sb[:], sgnc_d[:])
        biask_sb = sb.tile([128, 2], dt.float32, tag="biask")
        nc.scalar.dma_start(biask_sb[:], biask_d[:])

        # hash psum tiles: one [128, 512] tile per pair of chunks; chunk
        # 2g sits in the partition-block 0:64 and chunk 2g+1 in 64:128 of
        # the stacked input tile, and the block-diagonal wpk2 computes both
        # hashes in a single matmul pair (hi + lo): chunk 2g bits land at
        # psum rows 0:8, chunk 2g+1 bits at rows 64:72.
        hpk1a = hp.tile([128, 256], dt.float32, tag="hp", name="hpk1a")
        hpk1b = hp.tile([128, 256], dt.float32, tag="hp", name="hpk1b")
        hpk2 = hp.tile([128, 512], dt.float32, tag="hp", name="hpk2")

        # PE warm-up: anchor the p-state clock (a >~3us idle resets the PE
        # ramp). Garbage results land in hpk[0] rows that the real hash
        # matmuls overwrite with start=True.
        warm_sb = sb.tile([D, 64], dt.float16, tag="warm")
        nc.vector.memset(warm_sb[:], 0.0)
        for _ in range(2):
            nc.tensor.matmul(
                hpk1a[0:32, 0:64], lhsT=warm_sb[:, 0:32], rhs=warm_sb[:],
                start=True, stop=True,
            )

        def hash_pair(hpt, x2_ap):
            # x2 [128, n]: rows 0:64 = even chunk, 64:128 = odd chunk
            mm_hi = nc.tensor.matmul(
                hpt[:], lhsT=wpk2_sb[:, 0:128], rhs=x2_ap,
                start=True, stop=False,
            )
            nc.tensor.matmul(
                hpt[:], lhsT=wpk2_sb[:, 128:256], rhs=x2_ap,
                start=False, stop=True,
            )
            return mm_hi

        # ---- k side: bin (DVE) -> hash -> isgt 0/1 (DVE) -> agree ----
        xk2 = [
            sb.tile([128, 512], dt.float16, tag=f"xk2{g}", name=f"xk2{g}")
            for g in range(2)
        ]
        nc.vector.tensor_single_scalar(xk2[0][0:64, :], kT0_sb[:, 0:QTR], 0.0, Alu.is_gt)
        nc.vector.tensor_single_scalar(xk2[0][64:128, :], kT0_sb[:, QTR:HALF], 0.0, Alu.is_gt)
        nc.vector.tensor_single_scalar(xk2[1][0:64, :], kT1_sb[:, 0:QTR], 0.0, Alu.is_gt)
        nc.vector.tensor_single_scalar(xk2[1][64:128, :], kT1_sb[:, QTR:HALF], 0.0, Alu.is_gt)
        hash_pair(hpk1a, xk2[0][:, 0:256])
        hash_pair(hpk1b, xk2[0][:, 256:512])
        hash_pair(hpk2, xk2[1])

        # pair-1 bits as +-1 via ACT Sign in column-halves (ACT is free this
        # early; lets agree/relu/scan start sooner); pair-2 bits 0/1 via DVE
        s01k = sb.tile([128, 1024], dt.float16, tag="s01k")
        nc.scalar.activation(s01k[:, 0:256], hpk1a[:], Act.Sign)
        nc.scalar.activation(s01k[:, 256:512], hpk1b[:], Act.Sign)
        nc.vector.tensor_single_scalar(s01k[:, 512:1024], hpk2[:], 0.0, Alu.is_gt)

        # ---- q side: bin (Pool) -> hash -> sign +-1 (ACT halves) -> agree --
        xq2 = [
            sb.tile([128, 512], dt.float16, tag=f"xq2{g}", name=f"xq2{g}")
            for g in range(2)
        ]
        nc.gpsimd.tensor_single_scalar(xq2[0][0:64, :], qT_sb[:, 0:QTR], 0.0, Alu.is_gt)
        nc.gpsimd.tensor_single_scalar(xq2[0][64:128, :], qT_sb[:, QTR:HALF], 0.0, Alu.is_gt)
        nc.gpsimd.tensor_single_scalar(xq2[1][0:64, :], qT_sb[:, HALF : HALF + QTR], 0.0, Alu.is_gt)
        nc.gpsimd.tensor_single_scalar(xq2[1][64:128, :], qT_sb[:, HALF + QTR : L], 0.0, Alu.is_gt)

        # scatter data: each partition holds 0..L-1 (int16), off the
        # critical path (scat c0 needs it only after the first scan+mask)
        iota_sb = sb.tile([128, L], dt.float16, tag="iota")
        nc.gpsimd.iota(
            iota_sb[:], pattern=[[1, L]], base=1, channel_multiplier=0,
            allow_small_or_imprecise_dtypes=True,
        )

        onehot = sb.tile([128, L], dt.float16, tag="onehot")
        sq = sb.tile([128, 1024], dt.float16, tag="sq")
        q1h = sb.tile([128, 1536], dt.float16, tag="q1h")
        hpq = [hp.tile([128, 512], dt.float32, tag="hp", name=f"hpq{g}") for g in range(2)]

        def agree(rhs_ap, n, name):
            t = ap.tile([128, n], dt.float32, tag="apt", name=name)
            mm = nc.tensor.matmul(
                t[:],
                lhsT=sgnc_sb[rhs_ap.base_partition() : rhs_ap.base_partition() + 8, :],
                rhs=rhs_ap, start=True, stop=True,
            )
            return t, mm

        # PE emission order interleaves the q hash behind the k agrees so
        # the q chain keeps flowing while ACT works on the k relus.
        # k chunk 0 runs in column-halves so the scan chain starts early.
        from concourse.tile_rust import add_dep_helper

        apt_c0a, mm_c0a = agree(s01k[0:8, 0:256], 256, "apt_c0a")
        apt_c0b, mm_c0b = agree(s01k[0:8, 256:512], 256, "apt_c0b")
        apt_c1, mm_c1 = agree(s01k[64:72, 0:512], 512, "apt_c1")
        mm_hq0 = hash_pair(hpq[0], xq2[0])
        apt_c2, mm_c2 = agree(s01k[0:8, 512:1024], 512, "apt_c2")
        apt_c3, mm_c3 = agree(s01k[64:72, 512:1024], 512, "apt_c3")
        hash_pair(hpq[1], xq2[1])
        # keep the early k agrees (and the scan chain they feed) ahead of
        # everything later on PE
        add_dep_helper(mm_hq0.ins, mm_c1.ins, sync=False,
                       reason="chunk-0/1 agrees before q hash on PE")
        add_dep_helper(mm_c2.ins, mm_c1.ins, sync=False,
                       reason="chunk-0/1 agrees first on PE")

        # k one-hot: chunks 0,1 from +-1 signs (bias -7); 2,3 from 0/1 bits
        # (per-bucket bias 1-popcount)
        nc.scalar.activation(onehot[:, 0:256], apt_c0a[:], Act.Relu, bias=biask_sb[:, 1:2])
        nc.scalar.activation(onehot[:, 256:512], apt_c0b[:], Act.Relu, bias=biask_sb[:, 1:2])
        nc.scalar.activation(onehot[:, 512:1024], apt_c1[:], Act.Relu, bias=biask_sb[:, 1:2])
        nc.scalar.activation(onehot[:, 1024:1536], apt_c2[:], Act.Relu, bias=biask_sb[:, 0:1])
        nc.scalar.activation(onehot[:, 1536:2048], apt_c3[:], Act.Relu, bias=biask_sb[:, 0:1])
        nc.scalar.activation(sq[:, 0:512], hpq[0][:], Act.Sign)
        nc.scalar.activation(sq[:, 512:1024], hpq[1][:], Act.Sign)

        aptq = {}
        for u in range(4):
            r = 64 * (u % 2)
            g = u // 2
            aptq[u], _ = agree(sq[r : r + 8, 512 * g : 512 * (g + 1)], 512, f"aptq{u}")
        # q one-hot: chunks 0-2 on ACT, chunk 3 on DVE (after the scan chain)
        for u in range(3):
            nc.scalar.activation(
                q1h[:, 512 * u : 512 * (u + 1)], aptq[u][:],
                Act.Relu, bias=biask_sb[:, 1:2],
            )

        # ---- rank keys within bucket: quarter-granularity scan + mask.
        # m1 = onehot*rank is the 1-based rank at matches (0 elsewhere);
        # idx = m1 - 1 is the 0-based slot at matches, -1 (ignored) else.
        rank = sb.tile([128, L], dt.float16, tag="rank")
        m1 = sb.tile([128, L], dt.float16, tag="m1")
        idx16 = sb.tile([128, L], dt.int16, tag="idx16")
        pieces = [(0, 256), (256, 512), (512, 1024), (1024, 1536), (1536, 2048)]
        sub_inst = {}
        for i, (lo, hi) in enumerate(pieces):
            init = 0.0 if lo == 0 else rank[:, lo - 1 : lo]
            nc.vector.tensor_tensor_scan(
                rank[:, lo:hi], onehot[:, lo:hi], onehot[:, lo:hi],
                init, Alu.add, Alu.bypass,
            )
            nc.vector.tensor_mul(m1[:, lo:hi], onehot[:, lo:hi], rank[:, lo:hi])
            sub_inst[i] = nc.vector.tensor_single_scalar(
                idx16[:, lo:hi], m1[:, lo:hi], 1.0, Alu.subtract
            )
        tabs = []
        for c in range(4):
            lo, hi = QTR * c, QTR * (c + 1)
            tab = sb.tile([128, TABLE_ELEMS], dt.float16, tag=f"table{c}")
            tabs.append(tab)
            nc.gpsimd.local_scatter(
                tab[:], iota_sb[:, lo:hi], idx16[:, lo:hi],
                channels=128, num_elems=TABLE_ELEMS, num_idxs=QTR,
            )

        # q one-hot chunk 3 on DVE, held behind the scan chain so it does
        # not preempt the table build
        from concourse.tile_rust import add_dep_helper

        q1hx = sb.tile([128, 1024], dt.float16, tag="q1hx")
        aptq3_bf = aptq[3][:].bitcast(dt.bfloat16)
        q3_inst = nc.vector.tensor_single_scalar(q1hx[:], aptq3_bf, 7.0, Alu.is_gt)
        add_dep_helper(
            q3_inst.ins, sub_inst[4].ins, sync=False,
            reason="finish scan chain before q one-hot tail",
        )

        # merge quarter tables on Pool (disjoint nonzero slots); columns
        # 0..63 hold the first 64 matches (j+1) per bucket
        m01 = sb.tile([128, KMAX], dt.float16, tag="m01")
        nc.gpsimd.tensor_add(m01[:], tabs[0][:, 0:KMAX], tabs[1][:, 0:KMAX])
        m23 = sb.tile([128, KMAX], dt.float16, tag="m23")
        nc.gpsimd.tensor_add(m23[:], tabs[2][:, 0:KMAX], tabs[3][:, 0:KMAX])
        tab16 = sb.tile([128, KMAX], dt.float16, tag="tab16")
        nc.gpsimd.tensor_add(tab16[:], m01[:], m23[:])

        # ---- gather per query: out[i, s] = sum_c q1h[c, i] * tab16[c, s] ----
        # Chunk t takes queries {16p + t}, so psum partition p holds queries
        # 16p..16p+16 across chunks -> contiguous per-partition DRAM rows.
        q1hx_v = q1hx[:].rearrange("c (i two) -> c i two", two=2)[:, :, 1]
        HO = 8 * KMAX
        opA = gp.tile([128, HO], dt.float32, tag="gather", name="opA")
        opB = gp.tile([128, HO], dt.float32, tag="gatherB", name="opB")
        for t in range(16):
            dst = opA if t < 8 else opB
            if t < 12:
                lhsT = q1h[:, 128 * t : 128 * (t + 1)]
            else:
                lhsT = q1hx_v[:, 128 * (t - 12) : 128 * (t - 11)]
            nc.tensor.matmul(
                dst[:, KMAX * (t % 8) : KMAX * (t % 8 + 1)],
                lhsT=lhsT, rhs=tab16[:],
                start=True, stop=True,
            )
        out_v = out_d[:].rearrange("(p t) s -> p (t s)", p=128)  # [128, 1024] row-major view
        out0_sb = sb.tile([128, HO], dt.float16, tag="out0_sb")
        nc.vector.tensor_copy(out0_sb[:], opA[:])
        nc.sync.dma_start(out_v[:, 0:HO], out0_sb[:])
        out1_sb = sb.tile([128, HO], dt.float16, tag="out1_sb")
        nc.scalar.activation(out1_sb[:], opB[:], Act.Copy)
        nc.scalar.dma_start(out_v[:, HO : 2 * HO], out1_sb[:])

    nc.compile()
    return nc


def _get_nc():
    if "nc" not in _cache:
        _cache["nc"] = _build_program()
    return _cache["nc"]


def _make_in_maps(query, key, W):
    query = np.asarray(query, dtype=np.float32)
    key = np.asarray(key, dtype=np.float32)
    W = np.asarray(W, dtype=np.float32)
    qT = [
        np.ascontiguousarray(query[b].T).astype(ml_dtypes.bfloat16) for b in range(B)
    ]
    kT = [np.ascontiguousarray(key[b].T).astype(ml_dtypes.bfloat16) for b in range(B)]

    whi = W.astype(np.float16)
    wlo = (W - whi.astype(np.float32)).astype(np.float16)
    # block-diagonal pair weights: A window rows 0:64 -> cols 0:8 (+pad),
    # B window rows 64:128 -> cols 64:72; hi at cols 0:128, lo at 128:256
    wpk2 = np.zeros((128, 256), np.float16)
    wpk2[0:D, 0:NH] = whi
    wpk2[D : 2 * D, D : D + NH] = whi
    wpk2[0:D, 128 : 128 + NH] = wlo
    wpk2[D : 2 * D, 128 + D : 128 + D + NH] = wlo
    kT0 = [np.ascontiguousarray(kT[b][:, :HALF]) for b in range(B)]
    kT1 = [np.ascontiguousarray(kT[b][:, HALF:]) for b in range(B)]

    sgnc = []
    biask = []
    for h in range(2):
        cg = 128 * h + np.arange(128)  # global bucket ids of this half
        bits = ((cg[None, :] >> np.arange(NH)[:, None]) & 1).astype(np.float32)
        pm = (2.0 * bits - 1.0).astype(np.float16)  # [8, 128]
        arr = np.zeros((128, 128), np.float16)
        arr[0:NH] = pm
        arr[D : D + NH] = pm
        sgnc.append(arr)
        bk = np.empty((128, 2), np.float32)
        bk[:, 0] = 1.0 - bits.sum(axis=0)
        bk[:, 1] = -7.0
        biask.append(bk)
    return [
        {
            "qT": qT[c // 2],
            "kT0": kT0[c // 2],
            "kT1": kT1[c // 2],
            "wpk2": wpk2,
            "sgnc": sgnc[c % 2],
            "biask": biask[c % 2],
        }
        for c in range(2 * B)
    ]


def _combine(results):
    # device layout: [128, 16*64], partition p col t*64+s <-> query 128t+p
    out = np.empty((B, L, KMAX), dtype=np.int64)
    for b in range(B):
        g = results[2 * b]["out"].astype(np.int64) + results[2 * b + 1]["out"].astype(
            np.int64
        )
        g = g.reshape(128, 16, KMAX).transpose(1, 0, 2).reshape(L, KMAX)
        out[b] = g - 1
    return out


def _run_spmd(in_maps, **kwargs):
    from concourse.bass_utils import run_bass_kernel_spmd

    return run_bass_kernel_spmd(_get_nc(), in_maps, list(range(2 * B)), **kwargs)


def kernel(query, key, W, head_idx=0, **_unused):
    in_maps = _make_in_maps(query, key, W)
    res = _run_spmd(in_maps)
    return _combine(res.results)
